# revision 35
# baseline (speedup 1.0000x reference)
"""HGConv fused kernel for one TRN2 chip (8 NeuronCores), SPMD via Bass/Tile.

Hardcoded for M=16384 nodes, E=4096 hyperedges, D=300, N_CAT=3, 8 cores.

Edge-sharded design (v4c):
  - Core c owns hyperedges [512c, 512(c+1)).  It loads the FULL node
    features X (bf16, host-tiled) and its 512-column slice of inc
    (bf16, host-tiled), streaming both in m-blocks, and computes
    IX_c = inc[:, ec].T @ X  (512, 300) entirely locally -- no
    ReduceScatter (an 8-rank RS of the (4096,300) partials runs at
    ~31GB/s bus and costs ~150us; replicating the 9.8MB X read is far
    cheaper).
  - Weight/edge-feat loads are EMITTED AFTER the stream blocks on the
    same sync/scalar DMA queues, so their packets are serviced after
    the stream finishes (queue FIFO) instead of stealing engine time
    from it; each lands a couple of microseconds before its consumer.
  - Tail on the core's 512 edges, entirely in bf16 matmuls (the CPU
    quantization study shows phase-1 bf16 dominates the error budget;
    a bf16 tail moves the final rel err from 9.2e-3 to ~8e-3, far
    under the 2e-2 gate, while tripling PE column rate vs fp32):
      att = IX @ W_att (PE-transposed IX), softmax-over-d numerator
      ex = exp(att - max); Q = IX * ex is formed WITHOUT the 1/rowsum
      -- the reciprocal is folded into later per-edge scalars, so the
      Q transposes never wait for the accumulator readback.
      prjQ = Q @ ((1-a)*[W_proj | w2 | 0])  (host-prescaled, w2 =
      W_proj @ ec_W_att), then
      ef2 = rcp * prjQ[:, :300] + a*efeat   (one STT per e-tile)
      expw = exp(rcp * prjQ[:, 300] + a*sE) (one ACT per e-tile,
      scale/bias are per-partition columns) -- scores are O(1), no
      stabilization needed.
  - A ones-column is appended to ef2 so the weighted pool
    p2 = sum_e expw_e * [ef2_e | 1] also yields z = sum_e expw_e.
    Two PSUM accumulators (2 e-tiles each) halve the serial chain.
  - NO device collective: each core DMAs its 604-float partial
    [p2a | za | p2b | zb]; the host sums the 8 partials and applies
    the weights-only epilogue
    logits = (p2/z) @ ec_W_proj @ fc_W + (ec_b @ fc_W + fc_b).
  - att/proj matmuls are emitted chunk-major (e-tile fastest) so
    consecutive PE matmuls accumulate into different PSUM banks and
    pipeline at full column rate instead of draining between
    accumulation steps.
  - Dummy matmuls early in phase 1 keep the PE busy-looking while the
    stream ramps, so the activity throttle (HAM) never drops the clock
    to 4/8 (output is never read).
"""

import sys

for _p in ("/opt/trn_rl_repo", "/opt/pypackages"):
    if _p not in sys.path:
        sys.path.append(_p)

import numpy as np

import concourse.bacc as bacc
import concourse.tile as tile
from concourse import masks, mybir
from concourse.bass_utils import run_bass_kernel_spmd

F32 = mybir.dt.float32
BF16 = mybir.dt.bfloat16
AX = mybir.AxisListType
OP = mybir.AluOpType
AF = mybir.ActivationFunctionType

NCORES = 8
M, E, D, NCAT = 16384, 4096, 300, 3
E_SH = E // NCORES          # 512 edges per core
MT = M // 128               # 128 m-tiles (full node axis on every core)
ET = E_SH // 128            # 4 local e-tiles
DCH = (128, 128, 44)        # d split into partition chunks
DOF = (0, 128, 256)
# m-tiles per DMA block: small first blocks so the PE starts sooner
# during the DMA ramp, big steady-state blocks for packet efficiency
# (8-tile blocks = 13KB per-partition packets, ~5% more DMA headroom)
BLKS = (1, 1, 2, 2, 2, 4, 4, 4, 4) + (8,) * 13
NBLK = len(BLKS)
BOFF = [sum(BLKS[:i]) for i in range(NBLK)]
CW = E_SH + D               # combined [inc | x] row width per m-tile
DP1 = D + 2                 # proj width: w2 score col + even-pad
PKW = 4 * DP1               # per-core partial: 4x [p2_et|z_et]
NDUM = 6                   # blocks that get a HAM-damping dummy matmul


def _build(alpha: float, mode: str):
    nc = bacc.Bacc("TRN2", target_bir_lowering=False, debug=False,
                   num_devices=NCORES)
    in_dt = BF16 if mode == "bf16" else F32
    a = float(alpha)

    comb_d = nc.dram_tensor("combt", [128, MT, CW], in_dt,
                            kind="ExternalInput")
    watt_d = nc.dram_tensor("watt", [128, 3, D], BF16, kind="ExternalInput")
    wprojx_d = nc.dram_tensor("wprojx", [128, 3, DP1], BF16,
                              kind="ExternalInput")
    efs_d = nc.dram_tensor("efs", [128, ET, D + 1], BF16,
                           kind="ExternalInput")
    out_d = nc.dram_tensor("out", [1, PKW], F32, kind="ExternalOutput")

    def mm(out, lhsT, rhs, start, stop):
        nc.tensor.matmul(out, lhsT, rhs, start=start, stop=stop)

    with tile.TileContext(nc) as tc, \
         tc.tile_pool(name="sb", bufs=1) as sb:

        warm_sb = sb.tile([1, 64], BF16)
        # prime the sync/scalar DGE hardware queues so the first real
        # stream packet is not delayed by queue warmup
        nc.sync.dma_start(warm_sb[:, 0:32], watt_d[0:1, 0, 0:32])
        nc.scalar.dma_start(warm_sb[:, 32:64], watt_d[0:1, 0, 32:64])

        watt_sb = sb.tile([128, 3, D], BF16)
        wprojx_sb = sb.tile([128, 3, DP1], BF16)
        efs_sb = sb.tile([128, ET, D + 1], BF16)
        # host-pretiled partition-major: one contiguous chunk per
        # partition per tensor (128 packets each, no strided dribble);
        # efs col D carries a*sE (the exp bias column)
        nc.gpsimd.dma_start(watt_sb[:], watt_d.ap())
        nc.gpsimd.dma_start(wprojx_sb[:], wprojx_d.ap())
        nc.gpsimd.dma_start(efs_sb[:], efs_d.ap())

        # d axis padded to 384 so one XBAR DMA-transpose per e-tile
        # covers all three 128-row chunks (pad rows stay zero)
        ix_sb = sb.tile([128, ET, 384], BF16)
        ex_sb = sb.tile([128, ET, D], F32)
        q_sb = sb.tile([128, ET, 384], BF16)
        ixT_sb = sb.tile([128, 3, E_SH], BF16)
        qT_sb = sb.tile([128, 3, E_SH], BF16)
        nc.vector.memset(ix_sb[:, :, D:384], 0.0)
        nc.vector.memset(q_sb[:, :, D:384], 0.0)
        ef2_sb = sb.tile([128, ET, DP1], BF16)
        expcol_sb = sb.tile([128, ET], BF16)
        stat_sb = sb.tile([128, ET, 4], F32)
        pk_sb = sb.tile([1, PKW], F32)
        # ones-column so the p2 matmul also accumulates z = sum(expw)
        nc.vector.memset(ef2_sb[:, :, D:DP1], 1.0)

        with tc.tile_pool(name="ppix", bufs=1, space="PSUM") as ppix, \
             tc.tile_pool(name="ppd", bufs=1, space="PSUM") as ppd, \
             tc.tile_pool(name="xps", bufs=3) as xps, \
             tc.tile_pool(name="xp", bufs=7) as xp:

            # ---------- phase 1: IX = inc_cols.T @ X over all m ----------
            ixps = [ppix.tile([128, D], F32, tag=f"ix{ec}", name=f"ix{ec}")
                    for ec in range(ET)]
            for b in range(NBLK):
                blk = BLKS[b]
                pool = xps if blk < 8 else xp
                cb = pool.tile([128, blk, CW], in_dt,
                               tag=f"cb{blk}", name=f"cb{b}")
                eng = nc.sync if b % 2 == 0 else nc.scalar
                eng.dma_start(cb[:], comb_d[:, BOFF[b]:BOFF[b] + blk, :])
                for tl in range(blk):
                    mt = BOFF[b] + tl
                    for ec in range(ET):
                        mm(ixps[ec][:], cb[:, tl, ec * 128:(ec + 1) * 128],
                           cb[:, tl, E_SH:CW], start=(mt == 0),
                           stop=(mt == MT - 1))
                if b < NDUM:
                    # HAM damping: keep the PE busy-looking while the
                    # stream is DMA-bound, so the activity throttle never
                    # drops the clock to 4/8 (output is never read)
                    dmy = ppd.tile([128, 512], F32, tag="dmy", name="dmy")
                    nc.tensor.matmul(dmy[:], cb[:, 0, 0:128],
                                     cb[:, 0, 0:512], start=True, stop=True)

            # ---------- evacuate IX ----------
            for et in range(ET):
                if et % 2 == 0:
                    nc.vector.tensor_copy(ix_sb[:, et, 0:D], ixps[et][:])
                else:
                    nc.scalar.copy(ix_sb[:, et, 0:D], ixps[et][:])

        # ---------- tail on this core's 512 edges ----------
        with tc.tile_pool(name="ppd2", bufs=1, space="PSUM") as ppd2, \
             tc.tile_pool(name="ppm", bufs=1, space="PSUM") as ppm:
            # (ppd2 holds the HAM-damping dummy accumulators)

            TENG = (nc.sync, nc.scalar)

            # IX transpose: one XBAR DMA-transpose per e-tile (PE and
            # PSUM never touched; overlaps the last stream blocks)
            for et in range(ET):
                TENG[et % 2].dma_start(
                    ixT_sb[:, :, et * 128:(et + 1) * 128],
                    ix_sb[:, et, :], transpose=True)
            # back-to-back dummies fill the ~3us XBAR window so the
            # activity throttle holds 8/8 into the tail (never read)
            for k in range(8):
                dmy = ppd2.tile([128, 384], F32, tag=f"dmy{k % 2}")
                nc.tensor.matmul(dmy[:], ix_sb[:, k % ET, 0:128],
                                 ix_sb[:, k % ET, :], start=True, stop=True)

            # att = IX @ W_att, chunk-major so consecutive matmuls land in
            # different PSUM banks and pipeline at full column rate
            attps = [ppm.tile([128, D], F32, tag=f"mm{et}", name=f"att{et}")
                     for et in range(ET)]
            for et in range(ET):
                for i in range(3):
                    mm(attps[et][:], ixT_sb[:, i, et * 128:(et + 1) * 128],
                       watt_sb[:, i, :], start=(i == 0), stop=(i == 2))

            # softmax-over-d numerator; Q = IX * ex (1/rowsum deferred)
            for et in range(ET):
                att = attps[et]
                nmax = stat_sb[:, et, 0:1]
                nc.vector.tensor_reduce(nmax, att[:], axis=AX.X, op=OP.max,
                                        negate=True)
                rsum = stat_sb[:, et, 1:2]
                nc.scalar.activation(ex_sb[:, et, :], att[:], AF.Exp,
                                     bias=nmax, scale=1.0, accum_out=rsum)
                nc.vector.tensor_tensor(q_sb[:, et, 0:D], ex_sb[:, et, :],
                                        ix_sb[:, et, 0:D], op=OP.mult)
                rcp = stat_sb[:, et, 2:3]
                nc.vector.reciprocal(rcp, rsum)

            # Q transpose via XBAR (PE untouched); one per e-tile right
            # behind its softmax step, alternating the two HWDGE engines
            for et in range(ET):
                TENG[et % 2].dma_start(
                    qT_sb[:, :, et * 128:(et + 1) * 128],
                    q_sb[:, et, :], transpose=True)
                # HAM damping through the softmax-latency window
                dmy = ppd2.tile([128, 384], F32, tag="dmy0")
                nc.tensor.matmul(dmy[:], ix_sb[:, 0, 0:128],
                                 ix_sb[:, 0, :], start=True, stop=True)

            # prjQ = Q @ (1-a)[W_proj | w2 | 0]; col 300 is the score part
            # prj is emitted e-tile-major: each tile's chain completes as
            # soon as ITS transpose lands, so early tiles' ef2/expw run
            # while later tiles are still in softmax/transpose
            prjps = [ppm.tile([128, DP1], F32, tag=f"mm{et}", name=f"prj{et}")
                     for et in range(ET)]
            for et in range(ET):
                for i in range(3):
                    mm(prjps[et][:], qT_sb[:, i, et * 128:(et + 1) * 128],
                       wprojx_sb[:, i, :], start=(i == 0), stop=(i == 2))

            for et in range(ET):
                rcp = stat_sb[:, et, 2:3]
                # ef2 = alpha*efeat + rcp * (Q @ (1-a)W_proj)
                nc.vector.scalar_tensor_tensor(
                    ef2_sb[:, et, 0:D], prjps[et][:, 0:D], rcp,
                    efs_sb[:, et, 0:D], op0=OP.mult, op1=OP.add)
                # expw = exp(a*sE + rcp * (Q @ (1-a)w2)); scores are O(1),
                # no stabilization needed -- see module docstring
                nc.scalar.activation(expcol_sb[:, et:et + 1],
                                     prjps[et][:, D:D + 1], AF.Exp,
                                     bias=efs_sb[:, et, D:D + 1], scale=rcp)

            # p2_et = sum_e expw_e * [ef2_e | 1] per e-tile (col 300
            # accumulates z); four independent single-matmul accumulators
            # reuse the prj PSUM banks; the host sums the four partials
            for et in range(ET):
                p2 = ppm.tile([1, DP1], F32, tag=f"mm{et}", name=f"p2{et}")
                mm(p2[:], expcol_sb[:, et:et + 1], ef2_sb[:, et, :],
                   True, True)
                dmy = ppd2.tile([128, 384], F32, tag="dmy0")
                nc.tensor.matmul(dmy[:], ix_sb[:, 1, 0:128],
                                 ix_sb[:, 1, :], start=True, stop=True)
                if et % 2 == 0:
                    nc.scalar.copy(pk_sb[:, et * DP1:(et + 1) * DP1], p2[:])
                else:
                    nc.vector.tensor_copy(
                        pk_sb[:, et * DP1:(et + 1) * DP1], p2[:])
            nc.sync.dma_start(out_d[:, 0:2 * DP1], pk_sb[0:1, 0:2 * DP1])
            nc.sync.dma_start(out_d[:, 2 * DP1:PKW],
                              pk_sb[0:1, 2 * DP1:PKW])

    nc.compile()
    return nc


_CACHE = {}


def get_nc(alpha: float, mode: str = "bf16"):
    key = (alpha, mode)
    if key not in _CACHE:
        _CACHE[key] = _build(alpha, mode)
    return _CACHE[key]


def _tile_pm(arr2d):
    """(M, K) -> (128, M//128, K) with out[p, t, :] = arr[t*128 + p, :]."""
    mtot, k = arr2d.shape
    return np.ascontiguousarray(
        arr2d.reshape(mtot // 128, 128, k).swapaxes(0, 1))


def make_in_maps(node_feats, edge_feats, inc_mat, W_att, W_proj, alpha,
                 ec_W_att, mode="bf16"):
    import ml_dtypes
    bf = lambda x: np.ascontiguousarray(
        np.asarray(x, np.float32).astype(ml_dtypes.bfloat16))
    a = float(np.asarray(alpha))
    X = np.asarray(node_feats, np.float32)
    INC = np.asarray(inc_mat, np.float32)
    EF = np.asarray(edge_feats, np.float32)
    w2 = np.asarray(W_proj, np.float32) @ np.asarray(
        ec_W_att, np.float32).reshape(D, 1)            # (300, 1)
    wprojx = (1.0 - a) * np.concatenate(
        [np.asarray(W_proj, np.float32), w2, np.zeros((D, 1), np.float32)],
        axis=1)
    sE = EF @ np.asarray(ec_W_att, np.float32).reshape(D)   # (4096,)
    if mode == "bf16":
        X = X.astype(ml_dtypes.bfloat16)
        INC = INC.astype(ml_dtypes.bfloat16)
    xt = _tile_pm(X)
    # partition-major pretile: watt[p, i, :] = W_att[DOF[i] + p, :]
    # (rows beyond the 44-row chunk are padding; never read)
    def chunk_pm(w):
        out = np.zeros((128, 3, w.shape[1]), np.float32)
        for i, (c, o) in enumerate(zip(DCH, DOF)):
            out[:c, i, :] = w[o:o + c, :]
        return bf(out)
    common = dict(watt=chunk_pm(np.asarray(W_att, np.float32)),
                  wprojx=chunk_pm(wprojx))
    in_maps = []
    for c in range(NCORES):
        ef_sl = a * EF[c * E_SH:(c + 1) * E_SH]          # (512, 300)
        ase = (a * sE[c * E_SH:(c + 1) * E_SH]).reshape(ET, 128)
        efs = np.concatenate(
            [ef_sl.reshape(ET, 128, D), ase[:, :, None]],
            axis=2).swapaxes(0, 1)                       # (128, ET, 301)
        inct = _tile_pm(INC[:, c * E_SH:(c + 1) * E_SH])
        in_maps.append(dict(
            combt=np.ascontiguousarray(
                np.concatenate([inct, xt], axis=2)),
            efs=bf(efs),
            **common))
    return in_maps


def kernel(node_feats, edge_feats, inc_mat, W_att, W_proj, alpha,
           ec_W_att, ec_W_proj, ec_b_proj, fc_W, fc_b,
           mode="bf16", trace=False):
    nc = get_nc(float(np.asarray(alpha)), mode)
    in_maps = make_in_maps(node_feats, edge_feats, inc_mat, W_att, W_proj,
                           alpha, ec_W_att, mode=mode)
    res = run_bass_kernel_spmd(nc, in_maps, list(range(NCORES)), trace=trace)
    kernel.last_results = res
    pk = np.stack([np.asarray(r["out"], np.float64).reshape(PKW)
                   for r in res.results]).reshape(NCORES, 4, DP1)
    p2 = pk[:, :, 0:D].sum(axis=(0, 1))
    z = pk[:, :, D].sum()
    pooled = p2 / z
    out = pooled @ np.asarray(ec_W_proj, np.float64) + np.asarray(
        ec_b_proj, np.float64)
    logits = out @ np.asarray(fc_W, np.float64) + np.asarray(fc_b, np.float64)
    return logits.astype(np.float32)


# revision 36
# speedup vs baseline: 1.1282x; 1.1282x over previous
"""HGConv fused kernel for one TRN2 chip (8 NeuronCores), SPMD via Bass/Tile.

Hardcoded for M=16384 nodes, E=4096 hyperedges, D=300, N_CAT=3, 8 cores.

Edge-sharded design (v4c):
  - Core c owns hyperedges [512c, 512(c+1)).  It loads the FULL node
    features X (bf16, host-tiled) and its 512-column slice of inc
    (bf16, host-tiled), streaming both in m-blocks, and computes
    IX_c = inc[:, ec].T @ X  (512, 300) entirely locally -- no
    ReduceScatter (an 8-rank RS of the (4096,300) partials runs at
    ~31GB/s bus and costs ~150us; replicating the 9.8MB X read is far
    cheaper).
  - Weight/edge-feat loads are EMITTED AFTER the stream blocks on the
    same sync/scalar DMA queues, so their packets are serviced after
    the stream finishes (queue FIFO) instead of stealing engine time
    from it; each lands a couple of microseconds before its consumer.
  - Tail on the core's 512 edges, entirely in bf16 matmuls (the CPU
    quantization study shows phase-1 bf16 dominates the error budget;
    a bf16 tail moves the final rel err from 9.2e-3 to ~8e-3, far
    under the 2e-2 gate, while tripling PE column rate vs fp32):
      att = IX @ W_att (PE-transposed IX), softmax-over-d numerator
      ex = exp(att - max); Q = IX * ex is formed WITHOUT the 1/rowsum
      -- the reciprocal is folded into later per-edge scalars, so the
      Q transposes never wait for the accumulator readback.
      prjQ = Q @ ((1-a)*[W_proj | w2 | 0])  (host-prescaled, w2 =
      W_proj @ ec_W_att), then
      ef2 = rcp * prjQ[:, :300] + a*efeat   (one STT per e-tile)
      expw = exp(rcp * prjQ[:, 300] + a*sE) (one ACT per e-tile,
      scale/bias are per-partition columns) -- scores are O(1), no
      stabilization needed.
  - A ones-column is appended to ef2 so the weighted pool
    p2 = sum_e expw_e * [ef2_e | 1] also yields z = sum_e expw_e.
    Two PSUM accumulators (2 e-tiles each) halve the serial chain.
  - NO device collective: each core DMAs its 604-float partial
    [p2a | za | p2b | zb]; the host sums the 8 partials and applies
    the weights-only epilogue
    logits = (p2/z) @ ec_W_proj @ fc_W + (ec_b @ fc_W + fc_b).
  - att/proj matmuls are emitted chunk-major (e-tile fastest) so
    consecutive PE matmuls accumulate into different PSUM banks and
    pipeline at full column rate instead of draining between
    accumulation steps.
  - Dummy matmuls early in phase 1 keep the PE busy-looking while the
    stream ramps, so the activity throttle (HAM) never drops the clock
    to 4/8 (output is never read).
"""

import sys

for _p in ("/opt/trn_rl_repo", "/opt/pypackages"):
    if _p not in sys.path:
        sys.path.append(_p)

import numpy as np

import concourse.bacc as bacc
import concourse.tile as tile
from concourse import masks, mybir
from concourse.bass_utils import run_bass_kernel_spmd

F32 = mybir.dt.float32
BF16 = mybir.dt.bfloat16
AX = mybir.AxisListType
OP = mybir.AluOpType
AF = mybir.ActivationFunctionType

NCORES = 8
M, E, D, NCAT = 16384, 4096, 300, 3
E_SH = E // NCORES          # 512 edges per core
MT = M // 128               # 128 m-tiles (full node axis on every core)
ET = E_SH // 128            # 4 local e-tiles
DCH = (128, 128, 44)        # d split into partition chunks
DOF = (0, 128, 256)
# m-tiles per DMA block: small first blocks so the PE starts sooner
# during the DMA ramp, big steady-state blocks for packet efficiency
BLKS = (2, 2, 2, 2) + (4,) * 30
NBLK = len(BLKS)
BOFF = [sum(BLKS[:i]) for i in range(NBLK)]
CW = E_SH + D               # combined [inc | x] row width per m-tile
DP1 = D + 2                 # proj width: w2 score col + even-pad
PKW = 4 * DP1               # per-core partial: 4x [p2_et|z_et]
NDUM = 6                   # blocks that get a HAM-damping dummy matmul


def _build(alpha: float, mode: str):
    nc = bacc.Bacc("TRN2", target_bir_lowering=False, debug=False,
                   num_devices=NCORES)
    in_dt = BF16 if mode == "bf16" else F32
    a = float(alpha)

    comb_d = nc.dram_tensor("combt", [128, MT, CW], in_dt,
                            kind="ExternalInput")
    watt_d = nc.dram_tensor("watt", [128, 3, D], BF16, kind="ExternalInput")
    wprojx_d = nc.dram_tensor("wprojx", [128, 3, DP1], BF16,
                              kind="ExternalInput")
    efs_d = nc.dram_tensor("efs", [128, ET, D + 1], BF16,
                           kind="ExternalInput")
    out_d = nc.dram_tensor("out", [1, PKW], F32, kind="ExternalOutput")

    def mm(out, lhsT, rhs, start, stop):
        nc.tensor.matmul(out, lhsT, rhs, start=start, stop=stop)

    with tile.TileContext(nc) as tc, \
         tc.tile_pool(name="sb", bufs=1) as sb:

        warm_sb = sb.tile([1, 64], BF16)
        # prime the sync/scalar DGE hardware queues so the first real
        # stream packet is not delayed by queue warmup
        nc.sync.dma_start(warm_sb[:, 0:32], watt_d[0:1, 0, 0:32])
        nc.scalar.dma_start(warm_sb[:, 32:64], watt_d[0:1, 0, 32:64])

        watt_sb = sb.tile([128, 3, D], BF16)
        wprojx_sb = sb.tile([128, 3, DP1], BF16)
        efs_sb = sb.tile([128, ET, D + 1], BF16)
        # host-pretiled partition-major: one contiguous chunk per
        # partition per tensor (128 packets each, no strided dribble);
        # efs col D carries a*sE (the exp bias column)
        nc.gpsimd.dma_start(watt_sb[:], watt_d.ap())
        nc.gpsimd.dma_start(wprojx_sb[:], wprojx_d.ap())
        nc.gpsimd.dma_start(efs_sb[:], efs_d.ap())

        # d axis padded to 384 so one XBAR DMA-transpose per e-tile
        # covers all three 128-row chunks (pad rows stay zero)
        ix_sb = sb.tile([128, ET, 384], BF16)
        ex_sb = sb.tile([128, ET, D], F32)
        q_sb = sb.tile([128, ET, 384], BF16)
        ixT_sb = sb.tile([128, 3, E_SH], BF16)
        qT_sb = sb.tile([128, 3, E_SH], BF16)
        nc.vector.memset(ix_sb[:, :, D:384], 0.0)
        nc.vector.memset(q_sb[:, :, D:384], 0.0)
        ef2_sb = sb.tile([128, ET, DP1], BF16)
        expcol_sb = sb.tile([128, ET], BF16)
        stat_sb = sb.tile([128, ET, 4], F32)
        pk_sb = sb.tile([1, PKW], F32)
        # ones-column so the p2 matmul also accumulates z = sum(expw)
        nc.vector.memset(ef2_sb[:, :, D:DP1], 1.0)

        with tc.tile_pool(name="ppix", bufs=1, space="PSUM") as ppix, \
             tc.tile_pool(name="ppd", bufs=1, space="PSUM") as ppd, \
             tc.tile_pool(name="xps", bufs=4) as xps, \
             tc.tile_pool(name="xp", bufs=14) as xp:

            # ---------- phase 1: IX = inc_cols.T @ X over all m ----------
            ixps = [ppix.tile([128, D], F32, tag=f"ix{ec}", name=f"ix{ec}")
                    for ec in range(ET)]
            for b in range(NBLK):
                blk = BLKS[b]
                pool = xps if blk == 2 else xp
                cb = pool.tile([128, blk, CW], in_dt,
                               tag=f"cb{blk}", name=f"cb{b}")
                eng = nc.sync if b % 2 == 0 else nc.scalar
                eng.dma_start(cb[:], comb_d[:, BOFF[b]:BOFF[b] + blk, :])
                for tl in range(blk):
                    mt = BOFF[b] + tl
                    for ec in range(ET):
                        mm(ixps[ec][:], cb[:, tl, ec * 128:(ec + 1) * 128],
                           cb[:, tl, E_SH:CW], start=(mt == 0),
                           stop=(mt == MT - 1))
                if b < NDUM:
                    # HAM damping: keep the PE busy-looking while the
                    # stream is DMA-bound, so the activity throttle never
                    # drops the clock to 4/8 (output is never read)
                    dmy = ppd.tile([128, 512], F32, tag="dmy", name="dmy")
                    nc.tensor.matmul(dmy[:], cb[:, 0, 0:128],
                                     cb[:, 0, 0:512], start=True, stop=True)

            # ---------- evacuate IX ----------
            for et in range(ET):
                if et % 2 == 0:
                    nc.vector.tensor_copy(ix_sb[:, et, 0:D], ixps[et][:])
                else:
                    nc.scalar.copy(ix_sb[:, et, 0:D], ixps[et][:])

        # ---------- tail on this core's 512 edges ----------
        with tc.tile_pool(name="ppd2", bufs=1, space="PSUM") as ppd2, \
             tc.tile_pool(name="ppm", bufs=1, space="PSUM") as ppm:
            # (ppd2 holds the HAM-damping dummy accumulators)

            TENG = (nc.sync, nc.scalar)

            # IX transpose: one XBAR DMA-transpose per e-tile (PE and
            # PSUM never touched; overlaps the last stream blocks)
            for et in range(ET):
                TENG[et % 2].dma_start(
                    ixT_sb[:, :, et * 128:(et + 1) * 128],
                    ix_sb[:, et, :], transpose=True)
            # back-to-back dummies fill the ~3us XBAR window so the
            # activity throttle holds 8/8 into the tail (never read)
            for k in range(8):
                dmy = ppd2.tile([128, 384], F32, tag=f"dmy{k % 2}")
                nc.tensor.matmul(dmy[:], ix_sb[:, k % ET, 0:128],
                                 ix_sb[:, k % ET, :], start=True, stop=True)

            # att = IX @ W_att, chunk-major so consecutive matmuls land in
            # different PSUM banks and pipeline at full column rate
            attps = [ppm.tile([128, D], F32, tag=f"mm{et}", name=f"att{et}")
                     for et in range(ET)]
            for et in range(ET):
                for i in range(3):
                    mm(attps[et][:], ixT_sb[:, i, et * 128:(et + 1) * 128],
                       watt_sb[:, i, :], start=(i == 0), stop=(i == 2))

            # softmax-over-d numerator; Q = IX * ex (1/rowsum deferred)
            for et in range(ET):
                att = attps[et]
                nmax = stat_sb[:, et, 0:1]
                nc.vector.tensor_reduce(nmax, att[:], axis=AX.X, op=OP.max,
                                        negate=True)
                rsum = stat_sb[:, et, 1:2]
                nc.scalar.activation(ex_sb[:, et, :], att[:], AF.Exp,
                                     bias=nmax, scale=1.0, accum_out=rsum)
                nc.vector.tensor_tensor(q_sb[:, et, 0:D], ex_sb[:, et, :],
                                        ix_sb[:, et, 0:D], op=OP.mult)
                rcp = stat_sb[:, et, 2:3]
                nc.vector.reciprocal(rcp, rsum)

            # Q transpose via XBAR (PE untouched); one per e-tile right
            # behind its softmax step, alternating the two HWDGE engines
            for et in range(ET):
                TENG[et % 2].dma_start(
                    qT_sb[:, :, et * 128:(et + 1) * 128],
                    q_sb[:, et, :], transpose=True)
                # HAM damping through the softmax-latency window
                dmy = ppd2.tile([128, 384], F32, tag="dmy0")
                nc.tensor.matmul(dmy[:], ix_sb[:, 0, 0:128],
                                 ix_sb[:, 0, :], start=True, stop=True)

            # prjQ = Q @ (1-a)[W_proj | w2 | 0]; col 300 is the score part
            # prj is emitted e-tile-major: each tile's chain completes as
            # soon as ITS transpose lands, so early tiles' ef2/expw run
            # while later tiles are still in softmax/transpose
            prjps = [ppm.tile([128, DP1], F32, tag=f"mm{et}", name=f"prj{et}")
                     for et in range(ET)]
            for et in range(ET):
                for i in range(3):
                    mm(prjps[et][:], qT_sb[:, i, et * 128:(et + 1) * 128],
                       wprojx_sb[:, i, :], start=(i == 0), stop=(i == 2))

            for et in range(ET):
                rcp = stat_sb[:, et, 2:3]
                # ef2 = alpha*efeat + rcp * (Q @ (1-a)W_proj)
                nc.vector.scalar_tensor_tensor(
                    ef2_sb[:, et, 0:D], prjps[et][:, 0:D], rcp,
                    efs_sb[:, et, 0:D], op0=OP.mult, op1=OP.add)
                # expw = exp(a*sE + rcp * (Q @ (1-a)w2)); scores are O(1),
                # no stabilization needed -- see module docstring
                nc.scalar.activation(expcol_sb[:, et:et + 1],
                                     prjps[et][:, D:D + 1], AF.Exp,
                                     bias=efs_sb[:, et, D:D + 1], scale=rcp)

            # p2_et = sum_e expw_e * [ef2_e | 1] per e-tile (col 300
            # accumulates z); four independent single-matmul accumulators
            # reuse the prj PSUM banks; the host sums the four partials
            for et in range(ET):
                p2 = ppm.tile([1, DP1], F32, tag=f"mm{et}", name=f"p2{et}")
                mm(p2[:], expcol_sb[:, et:et + 1], ef2_sb[:, et, :],
                   True, True)
                dmy = ppd2.tile([128, 384], F32, tag="dmy0")
                nc.tensor.matmul(dmy[:], ix_sb[:, 1, 0:128],
                                 ix_sb[:, 1, :], start=True, stop=True)
                if et % 2 == 0:
                    nc.scalar.copy(pk_sb[:, et * DP1:(et + 1) * DP1], p2[:])
                else:
                    nc.vector.tensor_copy(
                        pk_sb[:, et * DP1:(et + 1) * DP1], p2[:])
            nc.sync.dma_start(out_d[:, 0:2 * DP1], pk_sb[0:1, 0:2 * DP1])
            nc.sync.dma_start(out_d[:, 2 * DP1:PKW],
                              pk_sb[0:1, 2 * DP1:PKW])

    nc.compile()
    return nc


_CACHE = {}


def get_nc(alpha: float, mode: str = "bf16"):
    key = (alpha, mode)
    if key not in _CACHE:
        _CACHE[key] = _build(alpha, mode)
    return _CACHE[key]


def _tile_pm(arr2d):
    """(M, K) -> (128, M//128, K) with out[p, t, :] = arr[t*128 + p, :]."""
    mtot, k = arr2d.shape
    return np.ascontiguousarray(
        arr2d.reshape(mtot // 128, 128, k).swapaxes(0, 1))


def make_in_maps(node_feats, edge_feats, inc_mat, W_att, W_proj, alpha,
                 ec_W_att, mode="bf16"):
    import ml_dtypes
    bf = lambda x: np.ascontiguousarray(
        np.asarray(x, np.float32).astype(ml_dtypes.bfloat16))
    a = float(np.asarray(alpha))
    X = np.asarray(node_feats, np.float32)
    INC = np.asarray(inc_mat, np.float32)
    EF = np.asarray(edge_feats, np.float32)
    w2 = np.asarray(W_proj, np.float32) @ np.asarray(
        ec_W_att, np.float32).reshape(D, 1)            # (300, 1)
    wprojx = (1.0 - a) * np.concatenate(
        [np.asarray(W_proj, np.float32), w2, np.zeros((D, 1), np.float32)],
        axis=1)
    sE = EF @ np.asarray(ec_W_att, np.float32).reshape(D)   # (4096,)
    if mode == "bf16":
        X = X.astype(ml_dtypes.bfloat16)
        INC = INC.astype(ml_dtypes.bfloat16)
    xt = _tile_pm(X)
    # partition-major pretile: watt[p, i, :] = W_att[DOF[i] + p, :]
    # (rows beyond the 44-row chunk are padding; never read)
    def chunk_pm(w):
        out = np.zeros((128, 3, w.shape[1]), np.float32)
        for i, (c, o) in enumerate(zip(DCH, DOF)):
            out[:c, i, :] = w[o:o + c, :]
        return bf(out)
    common = dict(watt=chunk_pm(np.asarray(W_att, np.float32)),
                  wprojx=chunk_pm(wprojx))
    in_maps = []
    for c in range(NCORES):
        ef_sl = a * EF[c * E_SH:(c + 1) * E_SH]          # (512, 300)
        ase = (a * sE[c * E_SH:(c + 1) * E_SH]).reshape(ET, 128)
        efs = np.concatenate(
            [ef_sl.reshape(ET, 128, D), ase[:, :, None]],
            axis=2).swapaxes(0, 1)                       # (128, ET, 301)
        inct = _tile_pm(INC[:, c * E_SH:(c + 1) * E_SH])
        in_maps.append(dict(
            combt=np.ascontiguousarray(
                np.concatenate([inct, xt], axis=2)),
            efs=bf(efs),
            **common))
    return in_maps


def kernel(node_feats, edge_feats, inc_mat, W_att, W_proj, alpha,
           ec_W_att, ec_W_proj, ec_b_proj, fc_W, fc_b,
           mode="bf16", trace=False):
    nc = get_nc(float(np.asarray(alpha)), mode)
    in_maps = make_in_maps(node_feats, edge_feats, inc_mat, W_att, W_proj,
                           alpha, ec_W_att, mode=mode)
    res = run_bass_kernel_spmd(nc, in_maps, list(range(NCORES)), trace=trace)
    kernel.last_results = res
    pk = np.stack([np.asarray(r["out"], np.float64).reshape(PKW)
                   for r in res.results]).reshape(NCORES, 4, DP1)
    p2 = pk[:, :, 0:D].sum(axis=(0, 1))
    z = pk[:, :, D].sum()
    pooled = p2 / z
    out = pooled @ np.asarray(ec_W_proj, np.float64) + np.asarray(
        ec_b_proj, np.float64)
    logits = out @ np.asarray(fc_W, np.float64) + np.asarray(fc_b, np.float64)
    return logits.astype(np.float32)
